# revision 21
# baseline (speedup 1.0000x reference)
"""Trainium2 Bass kernel for nn_CLUBCategorical (CLUB categorical loss).

Reference computation:
    h      = relu(x @ W1 + b1)              [N, H]
    logits = h @ W2 + b2                    [N, Y]
    logp   = log_softmax(logits, -1)        [N, Y]
    out[i] = logp[i, y_i] - mean_j logp[i, y_j]

Algebra: the log-softmax normalizer cancels between the positive and
negative terms, and with c[y] = histogram(y_idx), v = W2 @ c / N:

    out[i] = h_i . (W2[:, y_i] - v) + (b2[y_i] - (b2 . c)/N)
           = h_i . U[:, i] + g[i]

so the entire [N, H] x [H, Y] second GEMM collapses to an elementwise
multiply with the host-gathered U plus a free-dim reduction. Per core
(1024 rows) the device work is just the phase-1 GEMM:

    psum[128 rows, 512 h] = b1 (K=1 ones-matmul) + sum_k xT_blk @ W1_k
    hr = relu(psum)                     (scalar engine, bf16 out)
    delta[128,1] = reduce_h(hr * U_b) + g_b   (one fused DVE
                   tensor_tensor_reduce per 128-row block)

All matmul/elementwise operands are bf16 (PE runs 1 col/cycle at fp32r
and bf16 alike, but bf16 halves DMA to 4MB/core; tolerance is 2e-2 and
bf16 end-to-end lands ~5e-3). Rows are data-parallel across 8 cores; the
"all-gather of column labels" is folded into c/U/g on the host. No
collectives.

Schedule: the 8 sweep-1 bias matmuls are emitted before any data matmul
so the PE ramps its clock and does useful work during the DMA lead-in.
DMA rides two HWDGE queues (sync: cst/W1/U0-3/g, vector: x-blocks/U4-7)
ordered in consumption order.
"""

import numpy as np

N, X_DIM, Y_DIM, HIDDEN = 8192, 512, 512, 1024
N_CORES = 8
N_LOC = N // N_CORES          # 1024 rows per core
NB = N_LOC // 128             # 8 row blocks of 128
KX = X_DIM // 128             # 4 k-chunks
HH = 2                        # two 512-wide hidden halves

_NC_CACHE = {}


def _build(nc_cls, mybir, tile):
    mdt = mybir.dt
    f32 = mdt.float32
    bf16 = mdt.bfloat16
    AF = mybir.ActivationFunctionType
    OP = mybir.AluOpType

    nc = nc_cls("TRN2", target_bir_lowering=False, debug=False,
                num_devices=N_CORES)

    # DRAM tensors (all contiguous, one DMA descriptor each), sized so
    # each queue delivers in exact consumption order
    xq0D = nc.dram_tensor("xq0", [128, 512], bf16, kind="ExternalInput")
    xq12D = nc.dram_tensor("xq12", [128, 1024], bf16, kind="ExternalInput")
    xq345D = nc.dram_tensor("xq345", [128, 1536], bf16,
                            kind="ExternalInput")
    xq67D = nc.dram_tensor("xq67", [128, 1024], bf16, kind="ExternalInput")
    # w1 h-half 0 split per k-chunk; h-half 1 in one block
    w1kD = [nc.dram_tensor(f"w1k{k}", [128, 512], bf16,
                           kind="ExternalInput") for k in range(KX)]
    w1cD = nc.dram_tensor("w1c", [128, 2048], bf16, kind="ExternalInput")
    # U split by hidden half (h0 needed much earlier than h1)
    uhD = {(hh, i): nc.dram_tensor(f"u{hh}{i}", [128, 2048], bf16,
                                   kind="ExternalInput")
           for hh in range(HH) for i in range(2)}
    cstD = nc.dram_tensor("cst", [1, HIDDEN + 128], bf16,
                          kind="ExternalInput")   # [b1 | ones128]
    gtD = nc.dram_tensor("gt", [128, NB], f32, kind="ExternalInput")
    outD = nc.dram_tensor("out", [128, NB], f32, kind="ExternalOutput")

    with tile.TileContext(nc) as tc:
        with (
            tc.tile_pool(name="wgt", bufs=1) as wgt,
            tc.tile_pool(name="scrp", bufs=2) as scrp,
            tc.tile_pool(name="ps", bufs=6, space="PSUM") as ps,
            tc.tile_pool(name="psh", bufs=2, space="PSUM") as psh,
        ):
            cst_sb = wgt.tile([1, HIDDEN + 128], bf16, tag="cst")
            gt_sb = wgt.tile([128, NB], f32, tag="gt")
            wu = wgt.tile([128, 512], bf16, tag="wu")
            w1h = [wgt.tile([128, 2048], bf16, tag=f"w1h{h}", name=f"w1h{h}")
                   for h in range(HH)]
            xsb = wgt.tile([128, 4096], bf16, tag="xsb")
            # U by hidden half: uh[hh][:, b*512:(b+1)*512] = U half for blk b
            uh = [wgt.tile([128, 4096], bf16, tag=f"uh{h}", name=f"uh{h}")
                  for h in range(HH)]
            hr = [wgt.tile([128, 1024], bf16, tag=f"hr{b}", name=f"hr{b}")
                  for b in range(NB)]
            ra = wgt.tile([128, NB], f32, tag="ra")   # h0 partial + g
            outp = wgt.tile([128, NB], f32, tag="outp")
            xb = [xsb[:, b * 512:(b + 1) * 512] for b in range(NB)]

            onesQ = wgt.tile([128, 128], bf16, tag="onesQ")
            b1R = wgt.tile([128, 1024], bf16, tag="b1R")

            # DVE: warmup source + bias-broadcast constants (no DMA deps)
            nc.vector.memset(wu[:], 0.5)
            nc.vector.memset(onesQ[:], 1.0 / 128.0)
            # b1R[p, c] = b1[c], built by gpsimd off the critical path;
            # the K=128 bias matmul then computes sum_p b1[c]/128 = b1[c]
            nc.gpsimd.partition_broadcast(b1R[:], cst_sb[:, 0:HIDDEN], 128)

            # All DMA queues share ~320GB/s with racy arbitration, so the
            # layout keeps non-critical bytes (U, 2MB) behind the
            # PE-critical stream instead of competing with it.
            # scalar queue (q10): the PE-critical stream in exact
            # consumption order; out rides it at the end
            nc.scalar.dma_start(xsb[:, 0:512], xq0D.ap())
            nc.scalar.dma_start(w1h[0][:, 0:512], w1kD[0].ap())
            nc.scalar.dma_start(w1h[0][:, 1024:1536], w1kD[2].ap())
            nc.scalar.dma_start(xsb[:, 512:1536], xq12D.ap())
            nc.scalar.dma_start(xsb[:, 1536:3072], xq345D.ap())
            nc.scalar.dma_start(xsb[:, 3072:4096], xq67D.ap())
            nc.scalar.dma_start(w1h[1][:], w1cD.ap())

            # sync queue (q1, otherwise idle): cst first -- the bias MMs
            # need it earliest -- then the tiny g vector
            nc.sync.dma_start(cst_sb[:], cstD.ap())
            nc.sync.dma_start(gt_sb[:], gtD.ap())

            # gpsimd SWDGE: w1-h0 k1/k3 (early, small) then U halves.
            # The dummy copy makes the U triggers wait for the x stream --
            # without it the 2MB U burst starves the PE-critical bytes
            # (queues share bandwidth with racy arbitration).
            nc.gpsimd.dma_start(w1h[0][:, 512:1024], w1kD[1].ap())
            nc.gpsimd.dma_start(w1h[0][:, 1536:2048], w1kD[3].ap())
            # The scheduler orders by data deps only, so gate each U DMA
            # behind the x stream via a WAW stub write into its
            # destination: U-h0 waits for xq12, U-h1 for xq67.
            nc.gpsimd.tensor_copy(uh[0][:, 0:1], xsb[:, 1535:1536])
            nc.gpsimd.tensor_copy(uh[0][:, 2048:2049], xsb[:, 1535:1536])
            nc.gpsimd.tensor_copy(uh[1][:, 0:1], xsb[:, 4095:4096])
            nc.gpsimd.tensor_copy(uh[1][:, 2048:2049], xsb[:, 4095:4096])
            nc.gpsimd.dma_start(uh[0][:, 0:2048], uhD[(0, 0)].ap())
            nc.gpsimd.dma_start(uh[0][:, 2048:4096], uhD[(0, 1)].ap())
            nc.gpsimd.dma_start(uh[1][:, 0:2048], uhD[(1, 0)].ap())
            nc.gpsimd.dma_start(uh[1][:, 2048:4096], uhD[(1, 1)].ap())

            # PE warmup: no DMA deps -> runs right after the framework
            # barrier, ramping the PE p-state before real work lands
            wu_ps = ps.tile([128, 512], f32, tag="ps", bufs=6, name="wu_ps")
            for _ in range(4):
                nc.tensor.matmul(wu_ps[:], wu[:, 0:128], wu[:],
                                 start=True, stop=True)

            def bias_mm(psb, h):
                # psum[r, c] = sum_p b1[h*512+c]/128 = b1[h*512+c]
                nc.tensor.matmul(psb[:], onesQ[:],
                                 b1R[:, h * 512:(h + 1) * 512],
                                 start=True, stop=False)

            def data_mms(psb, b, h):
                for k in range(KX):
                    nc.tensor.matmul(
                        psb[:], xb[b][:, k * 128:(k + 1) * 128],
                        w1h[h][:, k * 512:(k + 1) * 512],
                        start=False, stop=(k == KX - 1))

            def dve_chain(b, h, acc_from, acc_to):
                # acc_to[:, b] = reduce_h(hr_half * U_half) + acc_from
                scr = scrp.tile([128, 512], bf16, tag="scr",
                                name=f"scr{h}_{b}")
                nc.vector.tensor_tensor(
                    scr[:], hr[b][:, h * 512:(h + 1) * 512],
                    uh[h][:, b * 512:(b + 1) * 512], OP.mult)
                red = scrp.tile([128, 1], f32, tag="red", name=f"red{h}_{b}")
                nc.vector.tensor_reduce(
                    red[:], scr[:], mybir.AxisListType.X, OP.add)
                nc.vector.tensor_tensor(
                    acc_to[:, b:b + 1], red[:], acc_from, OP.add)

            # sweep 1 (hidden half 0). Bias MMs 0-3 are emitted first --
            # they only need cst, so they fill the DMA lead-in window;
            # 4-7 are woven between data groups to plug w1/x-wait stalls.
            ps1 = [ps.tile([128, 512], f32, tag="ps", bufs=6,
                           name=f"ps0_{b}") for b in range(NB)]
            for b in range(4):
                bias_mm(ps1[b], 0)
            for b in range(NB):
                if b + 4 < NB:
                    bias_mm(ps1[b + 4], 0)
                data_mms(ps1[b], b, 0)
                nc.scalar.activation(hr[b][:, 0:512], ps1[b][:], AF.Relu)
                dve_chain(b, 0, gt_sb[:, b:b + 1], ra)

            # sweep 2 (hidden half 1): per-block bias+data+relu then the
            # closing DVE chain adds the h0 partial
            for b in range(NB - 1):
                psb = ps.tile([128, 512], f32, tag="ps", bufs=6,
                              name=f"ps1_{b}")
                bias_mm(psb, 1)
                data_mms(psb, b, 1)
                nc.scalar.activation(hr[b][:, 512:1024], psb[:], AF.Relu)
                dve_chain(b, 1, ra[:, b:b + 1], outp)

            # last block runs as two 256-wide half-groups so the closing
            # act/mult/reduce pipeline overlaps the final matmuls
            b = NB - 1
            t7 = wgt.tile([128, 1], f32, tag="t7")
            for hf in range(2):
                c0 = hf * 256
                psq = psh.tile([128, 256], f32, tag="psq", bufs=2,
                               name=f"psq{hf}")[:]
                nc.tensor.matmul(psq, onesQ[:],
                                 b1R[:, 512 + c0:512 + c0 + 256],
                                 start=True, stop=False)
                for k in range(KX):
                    nc.tensor.matmul(
                        psq, xb[b][:, k * 128:(k + 1) * 128],
                        w1h[1][:, k * 512 + c0:k * 512 + c0 + 256],
                        start=False, stop=(k == KX - 1))
                nc.scalar.activation(hr[b][:, 512 + c0:512 + c0 + 256],
                                     psq, AF.Relu)
                scr = scrp.tile([128, 256], bf16, tag="scrq",
                                name=f"scrq{hf}")
                nc.vector.tensor_tensor(
                    scr[:], hr[b][:, 512 + c0:512 + c0 + 256],
                    uh[1][:, b * 512 + c0:b * 512 + c0 + 256], OP.mult)
                red = scrp.tile([128, 1], f32, tag="redq", name=f"redq{hf}")
                nc.vector.tensor_reduce(
                    red[:], scr[:], mybir.AxisListType.X, OP.add)
                if hf == 0:
                    nc.vector.tensor_tensor(
                        t7[:], ra[:, b:b + 1], red[:], OP.add)
                else:
                    nc.vector.tensor_tensor(
                        outp[:, b:b + 1], t7[:], red[:], OP.add)

            nc.scalar.dma_start(outD.ap(), outp[:])

    nc.compile()
    return nc


def _get_nc():
    if "nc" not in _NC_CACHE:
        import concourse.bacc as bacc
        import concourse.mybir as mybir
        from concourse import tile
        _NC_CACHE["nc"] = _build(bacc.Bacc, mybir, tile)
    return _NC_CACHE["nc"]


def kernel(x_samples, y_idx, W1, b1, W2, b2):
    import ml_dtypes
    from concourse.bass_utils import run_bass_kernel_spmd

    bf16 = ml_dtypes.bfloat16
    x = np.ascontiguousarray(np.asarray(x_samples, dtype=np.float32))
    y = np.asarray(y_idx).astype(np.int64).reshape(-1)
    W1 = np.ascontiguousarray(np.asarray(W1, dtype=np.float32))
    b1 = np.asarray(b1, dtype=np.float32).reshape(-1)
    W2 = np.ascontiguousarray(np.asarray(W2, dtype=np.float32))
    b2 = np.asarray(b2, dtype=np.float32).reshape(-1)

    # global label histogram; fold the softmax-cancelled negative term
    c = np.bincount(y, minlength=Y_DIM).astype(np.float64)
    v = (W2.astype(np.float64) @ c / N).astype(np.float32)     # [H]
    beta = np.float32((b2.astype(np.float64) @ c) / N)
    g_full = (b2[y] - beta).astype(np.float32)                 # [N]

    # U columns, transposed: URt[i, :] = W2[:, y_i] - v
    W2pT = np.ascontiguousarray(W2.T - v[None, :])             # [Y, H]
    W2pT_bf = W2pT.astype(bf16)

    # W1 device layout (shared across cores): h-half 0 per k-chunk
    # (plain row slices), h-half 1 packed [p, k*512+c]
    W1_bf = W1.astype(bf16)
    w1k = [np.ascontiguousarray(W1_bf[k * 128:(k + 1) * 128, 0:512])
           for k in range(KX)]
    w1c = np.ascontiguousarray(
        W1_bf[:, 512:1024].reshape(KX, 128, 512)
        .transpose(1, 0, 2).reshape(128, 2048))
    cst = np.concatenate(
        [b1, np.ones(128, np.float32)]).astype(bf16).reshape(1, -1)

    x_bf = x.astype(bf16)
    in_maps = []
    for m in range(N_CORES):
        sl = slice(m * N_LOC, (m + 1) * N_LOC)
        y_loc = y[sl]
        ur = W2pT_bf[y_loc]                                    # [1024, H]
        im = {"w1c": w1c, "cst": cst,
              "gt": np.ascontiguousarray(
                  g_full[sl].reshape(NB, 128).T)}
        for k in range(KX):
            im[f"w1k{k}"] = w1k[k]
        # xs[p, b*512 + k*128 + r] = x[row0 + b*128 + r, k*128 + p]
        xs = np.ascontiguousarray(
            x_bf[sl].reshape(NB, 128, KX, 128)
            .transpose(3, 0, 2, 1).reshape(128, 4096))
        im["xq0"] = np.ascontiguousarray(xs[:, 0:512])
        im["xq12"] = np.ascontiguousarray(xs[:, 512:1536])
        im["xq345"] = np.ascontiguousarray(xs[:, 1536:3072])
        im["xq67"] = np.ascontiguousarray(xs[:, 3072:4096])
        # uh{hh}{i}[p, b*512 + c] = U[hh*512 + c, row0 + b*128 + p]
        for hh in range(HH):
            uu = np.ascontiguousarray(
                ur[:, hh * 512:(hh + 1) * 512].reshape(NB, 128, 512)
                .transpose(1, 0, 2).reshape(128, 4096))
            im[f"u{hh}0"] = np.ascontiguousarray(uu[:, 0:2048])
            im[f"u{hh}1"] = np.ascontiguousarray(uu[:, 2048:4096])
        in_maps.append(im)

    nc = _get_nc()
    res = run_bass_kernel_spmd(nc, in_maps, core_ids=list(range(N_CORES)))
    # out[p, blk] holds row blk*128+p of the core's 1024 rows
    return np.concatenate(
        [res.results[m]["out"].T.reshape(-1) for m in range(N_CORES)]
    ).astype(np.float32)


# revision 22
# speedup vs baseline: 1.1239x; 1.1239x over previous
"""Trainium2 Bass kernel for nn_CLUBCategorical (CLUB categorical loss).

Reference computation:
    h      = relu(x @ W1 + b1)              [N, H]
    logits = h @ W2 + b2                    [N, Y]
    logp   = log_softmax(logits, -1)        [N, Y]
    out[i] = logp[i, y_i] - mean_j logp[i, y_j]

Algebra: the log-softmax normalizer cancels between the positive and
negative terms, and with c[y] = histogram(y_idx), v = W2 @ c / N:

    out[i] = h_i . (W2[:, y_i] - v) + (b2[y_i] - (b2 . c)/N)
           = h_i . U[:, i] + g[i]

so the entire [N, H] x [H, Y] second GEMM collapses to an elementwise
multiply with the host-gathered U plus a free-dim reduction. Per core
(1024 rows) the device work is just the phase-1 GEMM:

    psum[128 rows, 512 h] = b1 (K=1 ones-matmul) + sum_k xT_blk @ W1_k
    hr = relu(psum)                     (scalar engine, bf16 out)
    delta[128,1] = reduce_h(hr * U_b) + g_b   (one fused DVE
                   tensor_tensor_reduce per 128-row block)

All matmul/elementwise operands are bf16 (PE runs 1 col/cycle at fp32r
and bf16 alike, but bf16 halves DMA to 4MB/core; tolerance is 2e-2 and
bf16 end-to-end lands ~5e-3). Rows are data-parallel across 8 cores; the
"all-gather of column labels" is folded into c/U/g on the host. No
collectives.

Schedule: the 8 sweep-1 bias matmuls are emitted before any data matmul
so the PE ramps its clock and does useful work during the DMA lead-in.
DMA rides two HWDGE queues (sync: cst/W1/U0-3/g, vector: x-blocks/U4-7)
ordered in consumption order.
"""

import numpy as np

N, X_DIM, Y_DIM, HIDDEN = 8192, 512, 512, 1024
N_CORES = 8
N_LOC = N // N_CORES          # 1024 rows per core
NB = N_LOC // 128             # 8 row blocks of 128
KX = X_DIM // 128             # 4 k-chunks
HH = 2                        # two 512-wide hidden halves

_NC_CACHE = {}


def _build(nc_cls, mybir, tile):
    mdt = mybir.dt
    f32 = mdt.float32
    bf16 = mdt.bfloat16
    AF = mybir.ActivationFunctionType
    OP = mybir.AluOpType

    nc = nc_cls("TRN2", target_bir_lowering=False, debug=False,
                num_devices=N_CORES)

    # DRAM tensors (all contiguous, one DMA descriptor each), sized so
    # each queue delivers in exact consumption order
    xq0D = nc.dram_tensor("xq0", [128, 512], bf16, kind="ExternalInput")
    xq12D = nc.dram_tensor("xq12", [128, 1024], bf16, kind="ExternalInput")
    xq345D = nc.dram_tensor("xq345", [128, 1536], bf16,
                            kind="ExternalInput")
    xq67D = nc.dram_tensor("xq67", [128, 1024], bf16, kind="ExternalInput")
    # w1 h-half 0 split per k-chunk; h-half 1 in one block
    w1kD = [nc.dram_tensor(f"w1k{k}", [128, 512], bf16,
                           kind="ExternalInput") for k in range(KX)]
    w1cD = nc.dram_tensor("w1c", [128, 2048], bf16, kind="ExternalInput")
    # U split by hidden half (h0 needed much earlier than h1)
    uhD = {(hh, i): nc.dram_tensor(f"u{hh}{i}", [128, 2048], bf16,
                                   kind="ExternalInput")
           for hh in range(HH) for i in range(2)}
    cstD = nc.dram_tensor("cst", [1, HIDDEN + 128], bf16,
                          kind="ExternalInput")   # [b1 | ones128]
    gtD = nc.dram_tensor("gt", [128, NB], f32, kind="ExternalInput")
    outD = nc.dram_tensor("out", [128, NB], f32, kind="ExternalOutput")

    with tile.TileContext(nc) as tc:
        with (
            tc.tile_pool(name="wgt", bufs=1) as wgt,
            tc.tile_pool(name="scrp", bufs=2) as scrp,
            tc.tile_pool(name="ps", bufs=6, space="PSUM") as ps,
            tc.tile_pool(name="psh", bufs=2, space="PSUM") as psh,
        ):
            cst_sb = wgt.tile([1, HIDDEN + 128], bf16, tag="cst")
            gt_sb = wgt.tile([128, NB], f32, tag="gt")
            wu = wgt.tile([128, 512], bf16, tag="wu")
            w1h = [wgt.tile([128, 2048], bf16, tag=f"w1h{h}", name=f"w1h{h}")
                   for h in range(HH)]
            xsb = wgt.tile([128, 4096], bf16, tag="xsb")
            # U by hidden half: uh[hh][:, b*512:(b+1)*512] = U half for blk b
            uh = [wgt.tile([128, 4096], bf16, tag=f"uh{h}", name=f"uh{h}")
                  for h in range(HH)]
            hr = [wgt.tile([128, 1024], bf16, tag=f"hr{b}", name=f"hr{b}")
                  for b in range(NB)]
            ra = wgt.tile([128, NB], f32, tag="ra")   # h0 partial + g
            outp = wgt.tile([128, NB], f32, tag="outp")
            xb = [xsb[:, b * 512:(b + 1) * 512] for b in range(NB)]

            onesQ = wgt.tile([128, 128], bf16, tag="onesQ")
            b1R = wgt.tile([128, 1024], bf16, tag="b1R")

            one128 = wgt.tile([1, 128], bf16, tag="one128")
            # DVE: warmup source + bias-broadcast constants (no DMA deps)
            nc.vector.memset(wu[:], 0.5)
            nc.vector.memset(one128[:], 1.0 / 128.0)
            nc.vector.memset(onesQ[:], 1.0)

            # All DMA queues share ~320GB/s with racy arbitration, so the
            # layout keeps non-critical bytes (U, 2MB) behind the
            # PE-critical stream instead of competing with it.
            # scalar queue (q10): the PE-critical stream in exact
            # consumption order; out rides it at the end
            nc.scalar.dma_start(xsb[:, 0:512], xq0D.ap())
            nc.scalar.dma_start(w1h[0][:, 0:512], w1kD[0].ap())
            nc.scalar.dma_start(w1h[0][:, 1024:1536], w1kD[2].ap())
            nc.scalar.dma_start(xsb[:, 512:1536], xq12D.ap())
            nc.scalar.dma_start(xsb[:, 1536:3072], xq345D.ap())
            nc.scalar.dma_start(xsb[:, 3072:4096], xq67D.ap())
            nc.scalar.dma_start(w1h[1][:], w1cD.ap())

            # sync queue (q1, otherwise idle): cst first -- the bias MMs
            # need it earliest -- then the tiny g vector
            nc.sync.dma_start(cst_sb[:], cstD.ap())
            nc.sync.dma_start(gt_sb[:], gtD.ap())

            # gpsimd SWDGE: w1-h0 k1/k3 (early, small) then U halves.
            # The dummy copy makes the U triggers wait for the x stream --
            # without it the 2MB U burst starves the PE-critical bytes
            # (queues share bandwidth with racy arbitration).
            nc.gpsimd.dma_start(w1h[0][:, 512:1024], w1kD[1].ap())
            nc.gpsimd.dma_start(w1h[0][:, 1536:2048], w1kD[3].ap())
            # The scheduler orders by data deps only, so gate each U DMA
            # behind the x stream via a WAW stub write into its
            # destination: U-h0 waits for xq12, U-h1 for xq67.
            nc.gpsimd.tensor_copy(uh[0][:, 0:1], xsb[:, 1535:1536])
            nc.gpsimd.tensor_copy(uh[0][:, 2048:2049], xsb[:, 1535:1536])
            nc.gpsimd.tensor_copy(uh[1][:, 0:1], xsb[:, 4095:4096])
            nc.gpsimd.tensor_copy(uh[1][:, 2048:2049], xsb[:, 4095:4096])
            nc.gpsimd.dma_start(uh[0][:, 0:2048], uhD[(0, 0)].ap())
            nc.gpsimd.dma_start(uh[0][:, 2048:4096], uhD[(0, 1)].ap())
            nc.gpsimd.dma_start(uh[1][:, 0:2048], uhD[(1, 0)].ap())
            nc.gpsimd.dma_start(uh[1][:, 2048:4096], uhD[(1, 1)].ap())

            # PE warmup: no DMA deps -> runs right after the framework
            # barrier, ramping the PE p-state before real work lands
            wu_ps = ps.tile([128, 512], f32, tag="ps", bufs=6, name="wu_ps")
            for _ in range(4):
                nc.tensor.matmul(wu_ps[:], wu[:, 0:128], wu[:],
                                 start=True, stop=True)

            # build b1R[p, c] = b1[c]/128 per half (K=1 matmul + DVE
            # copy) so the per-block bias init is a uniform K=128 matmul;
            # half 0 completes first so bias MMs unblock early
            for h in range(HH):
                psB = ps.tile([128, 512], f32, tag="ps", bufs=6,
                              name=f"b1b{h}")
                nc.tensor.matmul(psB[:], one128[:],
                                 cst_sb[:, h * 512:(h + 1) * 512],
                                 start=True, stop=True)
                nc.vector.tensor_copy(b1R[:, h * 512:(h + 1) * 512],
                                      psB[:])

            def bias_mm(psb, h):
                # psum[r, c] = sum_p b1[h*512+c]/128 = b1[h*512+c]
                nc.tensor.matmul(psb[:], onesQ[:],
                                 b1R[:, h * 512:(h + 1) * 512],
                                 start=True, stop=False)

            def data_mms(psb, b, h):
                for k in range(KX):
                    nc.tensor.matmul(
                        psb[:], xb[b][:, k * 128:(k + 1) * 128],
                        w1h[h][:, k * 512:(k + 1) * 512],
                        start=False, stop=(k == KX - 1))

            def dve_chain(b, h, acc_from, acc_to):
                # acc_to[:, b] = reduce_h(hr_half * U_half) + acc_from
                scr = scrp.tile([128, 512], bf16, tag="scr",
                                name=f"scr{h}_{b}")
                nc.vector.tensor_tensor(
                    scr[:], hr[b][:, h * 512:(h + 1) * 512],
                    uh[h][:, b * 512:(b + 1) * 512], OP.mult)
                red = scrp.tile([128, 1], f32, tag="red", name=f"red{h}_{b}")
                nc.vector.tensor_reduce(
                    red[:], scr[:], mybir.AxisListType.X, OP.add)
                nc.vector.tensor_tensor(
                    acc_to[:, b:b + 1], red[:], acc_from, OP.add)

            # sweep 1 (hidden half 0). Bias MMs 0-3 are emitted first --
            # they only need cst, so they fill the DMA lead-in window;
            # 4-7 are woven between data groups to plug w1/x-wait stalls.
            ps1 = [ps.tile([128, 512], f32, tag="ps", bufs=6,
                           name=f"ps0_{b}") for b in range(NB)]
            for b in range(4):
                bias_mm(ps1[b], 0)
            for b in range(NB):
                if b + 4 < NB:
                    bias_mm(ps1[b + 4], 0)
                data_mms(ps1[b], b, 0)
                nc.scalar.activation(hr[b][:, 0:512], ps1[b][:], AF.Relu)
                dve_chain(b, 0, gt_sb[:, b:b + 1], ra)

            # sweep 2 (hidden half 1): per-block bias+data+relu then the
            # closing DVE chain adds the h0 partial
            for b in range(NB - 1):
                psb = ps.tile([128, 512], f32, tag="ps", bufs=6,
                              name=f"ps1_{b}")
                bias_mm(psb, 1)
                data_mms(psb, b, 1)
                nc.scalar.activation(hr[b][:, 512:1024], psb[:], AF.Relu)
                dve_chain(b, 1, ra[:, b:b + 1], outp)

            # last block runs as two 256-wide half-groups so the closing
            # act/mult/reduce pipeline overlaps the final matmuls
            b = NB - 1
            t7 = wgt.tile([128, 1], f32, tag="t7")
            for hf in range(2):
                c0 = hf * 256
                psq = psh.tile([128, 256], f32, tag="psq", bufs=2,
                               name=f"psq{hf}")[:]
                nc.tensor.matmul(psq, onesQ[:],
                                 b1R[:, 512 + c0:512 + c0 + 256],
                                 start=True, stop=False)
                for k in range(KX):
                    nc.tensor.matmul(
                        psq, xb[b][:, k * 128:(k + 1) * 128],
                        w1h[1][:, k * 512 + c0:k * 512 + c0 + 256],
                        start=False, stop=(k == KX - 1))
                nc.scalar.activation(hr[b][:, 512 + c0:512 + c0 + 256],
                                     psq, AF.Relu)
                scr = scrp.tile([128, 256], bf16, tag="scrq",
                                name=f"scrq{hf}")
                nc.vector.tensor_tensor(
                    scr[:], hr[b][:, 512 + c0:512 + c0 + 256],
                    uh[1][:, b * 512 + c0:b * 512 + c0 + 256], OP.mult)
                red = scrp.tile([128, 1], f32, tag="redq", name=f"redq{hf}")
                nc.vector.tensor_reduce(
                    red[:], scr[:], mybir.AxisListType.X, OP.add)
                if hf == 0:
                    nc.vector.tensor_tensor(
                        t7[:], ra[:, b:b + 1], red[:], OP.add)
                else:
                    nc.vector.tensor_tensor(
                        outp[:, b:b + 1], t7[:], red[:], OP.add)

            nc.scalar.dma_start(outD.ap(), outp[:])

    nc.compile()
    return nc


def _get_nc():
    if "nc" not in _NC_CACHE:
        import concourse.bacc as bacc
        import concourse.mybir as mybir
        from concourse import tile
        _NC_CACHE["nc"] = _build(bacc.Bacc, mybir, tile)
    return _NC_CACHE["nc"]


def kernel(x_samples, y_idx, W1, b1, W2, b2):
    import ml_dtypes
    from concourse.bass_utils import run_bass_kernel_spmd

    bf16 = ml_dtypes.bfloat16
    x = np.ascontiguousarray(np.asarray(x_samples, dtype=np.float32))
    y = np.asarray(y_idx).astype(np.int64).reshape(-1)
    W1 = np.ascontiguousarray(np.asarray(W1, dtype=np.float32))
    b1 = np.asarray(b1, dtype=np.float32).reshape(-1)
    W2 = np.ascontiguousarray(np.asarray(W2, dtype=np.float32))
    b2 = np.asarray(b2, dtype=np.float32).reshape(-1)

    # global label histogram; fold the softmax-cancelled negative term
    c = np.bincount(y, minlength=Y_DIM).astype(np.float64)
    v = (W2.astype(np.float64) @ c / N).astype(np.float32)     # [H]
    beta = np.float32((b2.astype(np.float64) @ c) / N)
    g_full = (b2[y] - beta).astype(np.float32)                 # [N]

    # U columns, transposed: URt[i, :] = W2[:, y_i] - v
    W2pT = np.ascontiguousarray(W2.T - v[None, :])             # [Y, H]
    W2pT_bf = W2pT.astype(bf16)

    # W1 device layout (shared across cores): h-half 0 per k-chunk
    # (plain row slices), h-half 1 packed [p, k*512+c]
    W1_bf = W1.astype(bf16)
    w1k = [np.ascontiguousarray(W1_bf[k * 128:(k + 1) * 128, 0:512])
           for k in range(KX)]
    w1c = np.ascontiguousarray(
        W1_bf[:, 512:1024].reshape(KX, 128, 512)
        .transpose(1, 0, 2).reshape(128, 2048))
    cst = np.concatenate(
        [b1, np.ones(128, np.float32)]).astype(bf16).reshape(1, -1)

    x_bf = x.astype(bf16)
    in_maps = []
    for m in range(N_CORES):
        sl = slice(m * N_LOC, (m + 1) * N_LOC)
        y_loc = y[sl]
        ur = W2pT_bf[y_loc]                                    # [1024, H]
        im = {"w1c": w1c, "cst": cst,
              "gt": np.ascontiguousarray(
                  g_full[sl].reshape(NB, 128).T)}
        for k in range(KX):
            im[f"w1k{k}"] = w1k[k]
        # xs[p, b*512 + k*128 + r] = x[row0 + b*128 + r, k*128 + p]
        xs = np.ascontiguousarray(
            x_bf[sl].reshape(NB, 128, KX, 128)
            .transpose(3, 0, 2, 1).reshape(128, 4096))
        im["xq0"] = np.ascontiguousarray(xs[:, 0:512])
        im["xq12"] = np.ascontiguousarray(xs[:, 512:1536])
        im["xq345"] = np.ascontiguousarray(xs[:, 1536:3072])
        im["xq67"] = np.ascontiguousarray(xs[:, 3072:4096])
        # uh{hh}{i}[p, b*512 + c] = U[hh*512 + c, row0 + b*128 + p]
        for hh in range(HH):
            uu = np.ascontiguousarray(
                ur[:, hh * 512:(hh + 1) * 512].reshape(NB, 128, 512)
                .transpose(1, 0, 2).reshape(128, 4096))
            im[f"u{hh}0"] = np.ascontiguousarray(uu[:, 0:2048])
            im[f"u{hh}1"] = np.ascontiguousarray(uu[:, 2048:4096])
        in_maps.append(im)

    nc = _get_nc()
    res = run_bass_kernel_spmd(nc, in_maps, core_ids=list(range(N_CORES)))
    # out[p, blk] holds row blk*128+p of the core's 1024 rows
    return np.concatenate(
        [res.results[m]["out"].T.reshape(-1) for m in range(N_CORES)]
    ).astype(np.float32)


# revision 23
# speedup vs baseline: 1.2500x; 1.1122x over previous
"""Trainium2 Bass kernel for nn_CLUBCategorical (CLUB categorical loss).

Reference computation:
    h      = relu(x @ W1 + b1)              [N, H]
    logits = h @ W2 + b2                    [N, Y]
    logp   = log_softmax(logits, -1)        [N, Y]
    out[i] = logp[i, y_i] - mean_j logp[i, y_j]

Algebra: the log-softmax normalizer cancels between the positive and
negative terms, and with c[y] = histogram(y_idx), v = W2 @ c / N:

    out[i] = h_i . (W2[:, y_i] - v) + (b2[y_i] - (b2 . c)/N)
           = h_i . U[:, i] + g[i]

so the entire [N, H] x [H, Y] second GEMM collapses to an elementwise
multiply with the host-gathered U plus a free-dim reduction. Per core
(1024 rows) the device work is just the phase-1 GEMM:

    psum[128 rows, 512 h] = b1 (K=1 ones-matmul) + sum_k xT_blk @ W1_k
    hr = relu(psum)                     (scalar engine, bf16 out)
    delta[128,1] = reduce_h(hr * U_b) + g_b   (one fused DVE
                   tensor_tensor_reduce per 128-row block)

All matmul/elementwise operands are bf16 (PE runs 1 col/cycle at fp32r
and bf16 alike, but bf16 halves DMA to 4MB/core; tolerance is 2e-2 and
bf16 end-to-end lands ~5e-3). Rows are data-parallel across 8 cores; the
"all-gather of column labels" is folded into c/U/g on the host. No
collectives.

Schedule: the 8 sweep-1 bias matmuls are emitted before any data matmul
so the PE ramps its clock and does useful work during the DMA lead-in.
DMA rides two HWDGE queues (sync: cst/W1/U0-3/g, vector: x-blocks/U4-7)
ordered in consumption order.
"""

import numpy as np

N, X_DIM, Y_DIM, HIDDEN = 8192, 512, 512, 1024
N_CORES = 8
N_LOC = N // N_CORES          # 1024 rows per core
NB = N_LOC // 128             # 8 row blocks of 128
KX = X_DIM // 128             # 4 k-chunks
HH = 2                        # two 512-wide hidden halves

_NC_CACHE = {}


def _build(nc_cls, mybir, tile):
    mdt = mybir.dt
    f32 = mdt.float32
    bf16 = mdt.bfloat16
    AF = mybir.ActivationFunctionType
    OP = mybir.AluOpType

    nc = nc_cls("TRN2", target_bir_lowering=False, debug=False,
                num_devices=N_CORES)

    # DRAM tensors (all contiguous, one DMA descriptor each), sized so
    # each queue delivers in exact consumption order
    xq0D = nc.dram_tensor("xq0", [128, 512], bf16, kind="ExternalInput")
    xq12D = nc.dram_tensor("xq12", [128, 1024], bf16, kind="ExternalInput")
    xq345D = nc.dram_tensor("xq345", [128, 1536], bf16,
                            kind="ExternalInput")
    xq67D = nc.dram_tensor("xq67", [128, 1024], bf16, kind="ExternalInput")
    # w1 h-half 0 split per k-chunk; h-half 1 in one block
    w1kD = [nc.dram_tensor(f"w1k{k}", [128, 512], bf16,
                           kind="ExternalInput") for k in range(KX)]
    w1cD = nc.dram_tensor("w1c", [128, 2048], bf16, kind="ExternalInput")
    # U split by hidden half (h0 needed much earlier than h1)
    uhD = {(hh, i): nc.dram_tensor(f"u{hh}{i}", [128, 2048], bf16,
                                   kind="ExternalInput")
           for hh in range(HH) for i in range(2)}
    cstD = nc.dram_tensor("cst", [1, HIDDEN + 128], bf16,
                          kind="ExternalInput")   # [b1 | ones128]
    gtD = nc.dram_tensor("gt", [128, NB], f32, kind="ExternalInput")
    outD = nc.dram_tensor("out", [128, NB], f32, kind="ExternalOutput")

    with tile.TileContext(nc) as tc:
        with (
            tc.tile_pool(name="wgt", bufs=1) as wgt,
            tc.tile_pool(name="scrp", bufs=2) as scrp,
            tc.tile_pool(name="ps", bufs=7, space="PSUM") as ps,
            tc.tile_pool(name="psh", bufs=1, space="PSUM") as psh,
        ):
            cst_sb = wgt.tile([1, HIDDEN + 128], bf16, tag="cst")
            gt_sb = wgt.tile([128, NB], f32, tag="gt")
            wu = wgt.tile([128, 512], bf16, tag="wu")
            w1h = [wgt.tile([128, 2048], bf16, tag=f"w1h{h}", name=f"w1h{h}")
                   for h in range(HH)]
            xsb = wgt.tile([128, 4096], bf16, tag="xsb")
            # U by hidden half: uh[hh][:, b*512:(b+1)*512] = U half for blk b
            uh = [wgt.tile([128, 4096], bf16, tag=f"uh{h}", name=f"uh{h}")
                  for h in range(HH)]
            hr = [wgt.tile([128, 1024], bf16, tag=f"hr{b}", name=f"hr{b}")
                  for b in range(NB)]
            ra = wgt.tile([128, NB], f32, tag="ra")   # h0 partial + g
            outp = wgt.tile([128, NB], f32, tag="outp")
            xb = [xsb[:, b * 512:(b + 1) * 512] for b in range(NB)]

            onesQ = wgt.tile([128, 128], bf16, tag="onesQ")
            b1R = wgt.tile([128, 1024], bf16, tag="b1R")

            one128 = wgt.tile([1, 128], bf16, tag="one128")
            # DVE: warmup source + bias-broadcast constants (no DMA deps)
            nc.vector.memset(wu[:], 0.5)
            nc.vector.memset(one128[:], 1.0 / 128.0)
            nc.vector.memset(onesQ[:], 1.0)

            # All DMA queues share ~320GB/s with racy arbitration, so the
            # layout keeps non-critical bytes (U, 2MB) behind the
            # PE-critical stream instead of competing with it.
            # scalar queue (q10): the PE-critical stream in exact
            # consumption order; out rides it at the end
            nc.scalar.dma_start(xsb[:, 0:512], xq0D.ap())
            nc.scalar.dma_start(w1h[0][:, 0:512], w1kD[0].ap())
            nc.scalar.dma_start(w1h[0][:, 1024:1536], w1kD[2].ap())
            nc.scalar.dma_start(xsb[:, 512:1536], xq12D.ap())
            nc.scalar.dma_start(xsb[:, 1536:3072], xq345D.ap())
            nc.scalar.dma_start(xsb[:, 3072:4096], xq67D.ap())
            nc.scalar.dma_start(w1h[1][:], w1cD.ap())

            # sync queue (q1, otherwise idle): cst first -- the bias MMs
            # need it earliest -- then the tiny g vector
            nc.sync.dma_start(cst_sb[:], cstD.ap())
            nc.sync.dma_start(gt_sb[:], gtD.ap())

            # gpsimd SWDGE: w1-h0 k1/k3 (early, small) then U halves.
            # The dummy copy makes the U triggers wait for the x stream --
            # without it the 2MB U burst starves the PE-critical bytes
            # (queues share bandwidth with racy arbitration).
            nc.gpsimd.dma_start(w1h[0][:, 512:1024], w1kD[1].ap())
            nc.gpsimd.dma_start(w1h[0][:, 1536:2048], w1kD[3].ap())
            # The scheduler orders by data deps only, so gate each U DMA
            # behind the x stream via a WAW stub write into its
            # destination: U-h0 waits for xq12, U-h1 for xq67.
            nc.gpsimd.tensor_copy(uh[0][:, 0:1], xsb[:, 1535:1536])
            nc.gpsimd.tensor_copy(uh[0][:, 2048:2049], xsb[:, 1535:1536])
            nc.gpsimd.tensor_copy(uh[1][:, 0:1], xsb[:, 4095:4096])
            nc.gpsimd.tensor_copy(uh[1][:, 2048:2049], xsb[:, 4095:4096])
            nc.gpsimd.dma_start(uh[0][:, 0:2048], uhD[(0, 0)].ap())
            nc.gpsimd.dma_start(uh[0][:, 2048:4096], uhD[(0, 1)].ap())
            nc.gpsimd.dma_start(uh[1][:, 0:2048], uhD[(1, 0)].ap())
            nc.gpsimd.dma_start(uh[1][:, 2048:4096], uhD[(1, 1)].ap())

            # PE warmup: no DMA deps -> runs right after the framework
            # barrier, ramping the PE p-state before real work lands
            wu_ps = ps.tile([128, 512], f32, tag="ps", bufs=7, name="wu_ps")
            for _ in range(4):
                nc.tensor.matmul(wu_ps[:], wu[:, 0:128], wu[:],
                                 start=True, stop=True)

            # build b1R[p, c] = b1[c]/128 per half (K=1 matmul + copy)
            # so the per-block bias init is a uniform K=128 matmul
            for h in range(HH):
                psB = ps.tile([128, 512], f32, tag="ps", bufs=7,
                              name=f"b1b{h}")
                nc.tensor.matmul(psB[:], one128[:],
                                 cst_sb[:, h * 512:(h + 1) * 512],
                                 start=True, stop=True)
                nc.scalar.activation(b1R[:, h * 512:(h + 1) * 512], psB[:],
                                     AF.Copy)

            def bias_mm(psb, h):
                # psum[r, c] = sum_p b1[h*512+c]/128 = b1[h*512+c]
                nc.tensor.matmul(psb[:], onesQ[:],
                                 b1R[:, h * 512:(h + 1) * 512],
                                 start=True, stop=False)

            def data_mms(psb, b, h):
                for k in range(KX):
                    nc.tensor.matmul(
                        psb[:], xb[b][:, k * 128:(k + 1) * 128],
                        w1h[h][:, k * 512:(k + 1) * 512],
                        start=False, stop=(k == KX - 1))

            def dve_chain(b, h, acc_from, acc_to):
                # acc_to[:, b] = reduce_h(hr_half * U_half) + acc_from
                scr = scrp.tile([128, 512], bf16, tag="scr",
                                name=f"scr{h}_{b}")
                nc.vector.tensor_tensor(
                    scr[:], hr[b][:, h * 512:(h + 1) * 512],
                    uh[h][:, b * 512:(b + 1) * 512], OP.mult)
                red = scrp.tile([128, 1], f32, tag="red", name=f"red{h}_{b}")
                nc.vector.tensor_reduce(
                    red[:], scr[:], mybir.AxisListType.X, OP.add)
                nc.vector.tensor_tensor(
                    acc_to[:, b:b + 1], red[:], acc_from, OP.add)

            # sweep 1 (hidden half 0). Bias MMs 0-3 are emitted first --
            # they only need cst, so they fill the DMA lead-in window;
            # 4-7 are woven between data groups to plug w1/x-wait stalls.
            ps1 = [ps.tile([128, 512], f32, tag="ps", bufs=7,
                           name=f"ps0_{b}") for b in range(NB)]
            for b in range(4):
                bias_mm(ps1[b], 0)
            for b in range(NB):
                if b + 4 < NB:
                    bias_mm(ps1[b + 4], 0)
                data_mms(ps1[b], b, 0)
                nc.scalar.activation(hr[b][:, 0:512], ps1[b][:], AF.Relu)
                dve_chain(b, 0, gt_sb[:, b:b + 1], ra)

            # sweep 2 (hidden half 1): per-block bias+data+relu then the
            # closing DVE chain adds the h0 partial
            for b in range(NB - 1):
                psb = ps.tile([128, 512], f32, tag="ps", bufs=7,
                              name=f"ps1_{b}")
                bias_mm(psb, 1)
                data_mms(psb, b, 1)
                nc.scalar.activation(hr[b][:, 512:1024], psb[:], AF.Relu)
                dve_chain(b, 1, ra[:, b:b + 1], outp)

            # last block runs as two 256-wide half-groups so the closing
            # act/mult/reduce pipeline overlaps the final matmuls
            b = NB - 1
            t7 = wgt.tile([128, 1], f32, tag="t7")
            psq2 = psh.tile([128, 512], f32, tag="psq", name="psq")
            for hf in range(2):
                c0 = hf * 256
                psq = psq2[:, c0:c0 + 256]
                nc.tensor.matmul(psq, onesQ[:],
                                 b1R[:, 512 + c0:512 + c0 + 256],
                                 start=True, stop=False)
                for k in range(KX):
                    nc.tensor.matmul(
                        psq, xb[b][:, k * 128:(k + 1) * 128],
                        w1h[1][:, k * 512 + c0:k * 512 + c0 + 256],
                        start=False, stop=(k == KX - 1))
                nc.scalar.activation(hr[b][:, 512 + c0:512 + c0 + 256],
                                     psq, AF.Relu)
                scr = scrp.tile([128, 256], bf16, tag="scrq",
                                name=f"scrq{hf}")
                nc.vector.tensor_tensor(
                    scr[:], hr[b][:, 512 + c0:512 + c0 + 256],
                    uh[1][:, b * 512 + c0:b * 512 + c0 + 256], OP.mult)
                red = scrp.tile([128, 1], f32, tag="redq", name=f"redq{hf}")
                nc.vector.tensor_reduce(
                    red[:], scr[:], mybir.AxisListType.X, OP.add)
                if hf == 0:
                    nc.vector.tensor_tensor(
                        t7[:], ra[:, b:b + 1], red[:], OP.add)
                else:
                    nc.vector.tensor_tensor(
                        outp[:, b:b + 1], t7[:], red[:], OP.add)

            nc.scalar.dma_start(outD.ap(), outp[:])

    nc.compile()
    return nc


def _get_nc():
    if "nc" not in _NC_CACHE:
        import concourse.bacc as bacc
        import concourse.mybir as mybir
        from concourse import tile
        _NC_CACHE["nc"] = _build(bacc.Bacc, mybir, tile)
    return _NC_CACHE["nc"]


def kernel(x_samples, y_idx, W1, b1, W2, b2):
    import ml_dtypes
    from concourse.bass_utils import run_bass_kernel_spmd

    bf16 = ml_dtypes.bfloat16
    x = np.ascontiguousarray(np.asarray(x_samples, dtype=np.float32))
    y = np.asarray(y_idx).astype(np.int64).reshape(-1)
    W1 = np.ascontiguousarray(np.asarray(W1, dtype=np.float32))
    b1 = np.asarray(b1, dtype=np.float32).reshape(-1)
    W2 = np.ascontiguousarray(np.asarray(W2, dtype=np.float32))
    b2 = np.asarray(b2, dtype=np.float32).reshape(-1)

    # global label histogram; fold the softmax-cancelled negative term
    c = np.bincount(y, minlength=Y_DIM).astype(np.float64)
    v = (W2.astype(np.float64) @ c / N).astype(np.float32)     # [H]
    beta = np.float32((b2.astype(np.float64) @ c) / N)
    g_full = (b2[y] - beta).astype(np.float32)                 # [N]

    # U columns, transposed: URt[i, :] = W2[:, y_i] - v
    W2pT = np.ascontiguousarray(W2.T - v[None, :])             # [Y, H]
    W2pT_bf = W2pT.astype(bf16)

    # W1 device layout (shared across cores): h-half 0 per k-chunk
    # (plain row slices), h-half 1 packed [p, k*512+c]
    W1_bf = W1.astype(bf16)
    w1k = [np.ascontiguousarray(W1_bf[k * 128:(k + 1) * 128, 0:512])
           for k in range(KX)]
    w1c = np.ascontiguousarray(
        W1_bf[:, 512:1024].reshape(KX, 128, 512)
        .transpose(1, 0, 2).reshape(128, 2048))
    cst = np.concatenate(
        [b1, np.ones(128, np.float32)]).astype(bf16).reshape(1, -1)

    x_bf = x.astype(bf16)
    in_maps = []
    for m in range(N_CORES):
        sl = slice(m * N_LOC, (m + 1) * N_LOC)
        y_loc = y[sl]
        ur = W2pT_bf[y_loc]                                    # [1024, H]
        im = {"w1c": w1c, "cst": cst,
              "gt": np.ascontiguousarray(
                  g_full[sl].reshape(NB, 128).T)}
        for k in range(KX):
            im[f"w1k{k}"] = w1k[k]
        # xs[p, b*512 + k*128 + r] = x[row0 + b*128 + r, k*128 + p]
        xs = np.ascontiguousarray(
            x_bf[sl].reshape(NB, 128, KX, 128)
            .transpose(3, 0, 2, 1).reshape(128, 4096))
        im["xq0"] = np.ascontiguousarray(xs[:, 0:512])
        im["xq12"] = np.ascontiguousarray(xs[:, 512:1536])
        im["xq345"] = np.ascontiguousarray(xs[:, 1536:3072])
        im["xq67"] = np.ascontiguousarray(xs[:, 3072:4096])
        # uh{hh}{i}[p, b*512 + c] = U[hh*512 + c, row0 + b*128 + p]
        for hh in range(HH):
            uu = np.ascontiguousarray(
                ur[:, hh * 512:(hh + 1) * 512].reshape(NB, 128, 512)
                .transpose(1, 0, 2).reshape(128, 4096))
            im[f"u{hh}0"] = np.ascontiguousarray(uu[:, 0:2048])
            im[f"u{hh}1"] = np.ascontiguousarray(uu[:, 2048:4096])
        in_maps.append(im)

    nc = _get_nc()
    res = run_bass_kernel_spmd(nc, in_maps, core_ids=list(range(N_CORES)))
    # out[p, blk] holds row blk*128+p of the core's 1024 rows
    return np.concatenate(
        [res.results[m]["out"].T.reshape(-1) for m in range(N_CORES)]
    ).astype(np.float32)


# revision 24
# speedup vs baseline: 1.2707x; 1.0166x over previous
"""Trainium2 Bass kernel for nn_CLUBCategorical (CLUB categorical loss).

Reference computation:
    h      = relu(x @ W1 + b1)              [N, H]
    logits = h @ W2 + b2                    [N, Y]
    logp   = log_softmax(logits, -1)        [N, Y]
    out[i] = logp[i, y_i] - mean_j logp[i, y_j]

Algebra: the log-softmax normalizer cancels between the positive and
negative terms, and with c[y] = histogram(y_idx), v = W2 @ c / N:

    out[i] = h_i . (W2[:, y_i] - v) + (b2[y_i] - (b2 . c)/N)
           = h_i . U[:, i] + g[i]

so the entire [N, H] x [H, Y] second GEMM collapses to an elementwise
multiply with the host-gathered U plus a free-dim reduction. Per core
(1024 rows) the device work is just the phase-1 GEMM:

    psum[128 rows, 512 h] = b1 (K=1 ones-matmul) + sum_k xT_blk @ W1_k
    hr = relu(psum)                     (scalar engine, bf16 out)
    delta[128,1] = reduce_h(hr * U_b) + g_b   (one fused DVE
                   tensor_tensor_reduce per 128-row block)

All matmul/elementwise operands are bf16 (PE runs 1 col/cycle at fp32r
and bf16 alike, but bf16 halves DMA to 4MB/core; tolerance is 2e-2 and
bf16 end-to-end lands ~5e-3). Rows are data-parallel across 8 cores; the
"all-gather of column labels" is folded into c/U/g on the host. No
collectives.

Schedule: the 8 sweep-1 bias matmuls are emitted before any data matmul
so the PE ramps its clock and does useful work during the DMA lead-in.
DMA rides two HWDGE queues (sync: cst/W1/U0-3/g, vector: x-blocks/U4-7)
ordered in consumption order.
"""

import numpy as np

N, X_DIM, Y_DIM, HIDDEN = 8192, 512, 512, 1024
N_CORES = 8
N_LOC = N // N_CORES          # 1024 rows per core
NB = N_LOC // 128             # 8 row blocks of 128
KX = X_DIM // 128             # 4 k-chunks
HH = 2                        # two 512-wide hidden halves

_NC_CACHE = {}


def _build(nc_cls, mybir, tile):
    mdt = mybir.dt
    f32 = mdt.float32
    bf16 = mdt.bfloat16
    AF = mybir.ActivationFunctionType
    OP = mybir.AluOpType

    nc = nc_cls("TRN2", target_bir_lowering=False, debug=False,
                num_devices=N_CORES)

    # DRAM tensors (all contiguous, one DMA descriptor each), sized so
    # each queue delivers in exact consumption order
    xq0D = nc.dram_tensor("xq0", [128, 512], bf16, kind="ExternalInput")
    xq12D = nc.dram_tensor("xq12", [128, 1024], bf16, kind="ExternalInput")
    xq345D = nc.dram_tensor("xq345", [128, 1536], bf16,
                            kind="ExternalInput")
    xq67D = nc.dram_tensor("xq67", [128, 1024], bf16, kind="ExternalInput")
    # w1 h-half 0 split per k-chunk; h-half 1 in one block
    w1kD = [nc.dram_tensor(f"w1k{k}", [128, 512], bf16,
                           kind="ExternalInput") for k in range(KX)]
    w1cD = nc.dram_tensor("w1c", [128, 2048], bf16, kind="ExternalInput")
    # U split by hidden half (h0 needed much earlier than h1)
    uhD = {(hh, i): nc.dram_tensor(f"u{hh}{i}", [128, 2048], bf16,
                                   kind="ExternalInput")
           for hh in range(HH) for i in range(2)}
    cstD = nc.dram_tensor("cst", [1, HIDDEN + 128], bf16,
                          kind="ExternalInput")   # [b1 | ones128]
    gtD = nc.dram_tensor("gt", [128, NB], f32, kind="ExternalInput")
    outD = nc.dram_tensor("out", [128, NB], f32, kind="ExternalOutput")

    with tile.TileContext(nc) as tc:
        with (
            tc.tile_pool(name="wgt", bufs=1) as wgt,
            tc.tile_pool(name="scrp", bufs=2) as scrp,
            tc.tile_pool(name="ps", bufs=7, space="PSUM") as ps,
            tc.tile_pool(name="psh", bufs=1, space="PSUM") as psh,
        ):
            cst_sb = wgt.tile([1, HIDDEN + 128], bf16, tag="cst")
            gt_sb = wgt.tile([128, NB], f32, tag="gt")
            wu = wgt.tile([128, 512], bf16, tag="wu")
            w1h = [wgt.tile([128, 2048], bf16, tag=f"w1h{h}", name=f"w1h{h}")
                   for h in range(HH)]
            xsb = wgt.tile([128, 4096], bf16, tag="xsb")
            # U by hidden half: uh[hh][:, b*512:(b+1)*512] = U half for blk b
            uh = [wgt.tile([128, 4096], bf16, tag=f"uh{h}", name=f"uh{h}")
                  for h in range(HH)]
            hr = [wgt.tile([128, 1024], bf16, tag=f"hr{b}", name=f"hr{b}")
                  for b in range(NB)]
            ra = wgt.tile([128, NB], f32, tag="ra")   # h0 partial + g
            outp = wgt.tile([128, NB], f32, tag="outp")
            xb = [xsb[:, b * 512:(b + 1) * 512] for b in range(NB)]

            onesQ = wgt.tile([128, 128], bf16, tag="onesQ")
            b1R = wgt.tile([128, 1024], bf16, tag="b1R")

            one128 = wgt.tile([1, 128], bf16, tag="one128")
            # DVE: warmup source + bias-broadcast constants (no DMA deps)
            nc.vector.memset(wu[:], 0.5)
            nc.vector.memset(one128[:], 1.0 / 128.0)
            nc.vector.memset(onesQ[:], 1.0)

            # All DMA queues share ~320GB/s with racy arbitration, so the
            # layout keeps non-critical bytes (U, 2MB) behind the
            # PE-critical stream instead of competing with it.
            # scalar queue (q10): the PE-critical stream in exact
            # consumption order; out rides it at the end
            nc.scalar.dma_start(xsb[:, 0:512], xq0D.ap())
            nc.scalar.dma_start(w1h[0][:, 0:512], w1kD[0].ap())
            nc.scalar.dma_start(w1h[0][:, 1024:1536], w1kD[2].ap())
            nc.scalar.dma_start(xsb[:, 512:1536], xq12D.ap())
            nc.scalar.dma_start(xsb[:, 1536:3072], xq345D.ap())
            nc.scalar.dma_start(xsb[:, 3072:4096], xq67D.ap())
            nc.scalar.dma_start(w1h[1][:], w1cD.ap())

            # sync queue (q1, otherwise idle): cst first -- the bias MMs
            # need it earliest -- then the tiny g vector
            nc.sync.dma_start(cst_sb[:], cstD.ap())
            nc.sync.dma_start(gt_sb[:], gtD.ap())

            # gpsimd SWDGE: w1-h0 k1/k3 (early, small) then U halves.
            # The dummy copy makes the U triggers wait for the x stream --
            # without it the 2MB U burst starves the PE-critical bytes
            # (queues share bandwidth with racy arbitration).
            nc.gpsimd.dma_start(w1h[0][:, 512:1024], w1kD[1].ap())
            nc.gpsimd.dma_start(w1h[0][:, 1536:2048], w1kD[3].ap())
            # The scheduler orders by data deps only, so gate each U DMA
            # behind the x stream via a WAW stub write into its
            # destination: U-h0 waits for xq12, U-h1 for xq67.
            nc.gpsimd.tensor_copy(uh[0][:, 0:1], xsb[:, 1535:1536])
            nc.gpsimd.tensor_copy(uh[0][:, 2048:2049], xsb[:, 1535:1536])
            nc.gpsimd.tensor_copy(uh[1][:, 0:1], xsb[:, 4095:4096])
            nc.gpsimd.tensor_copy(uh[1][:, 2048:2049], xsb[:, 4095:4096])
            nc.gpsimd.dma_start(uh[0][:, 0:2048], uhD[(0, 0)].ap())
            nc.gpsimd.dma_start(uh[0][:, 2048:4096], uhD[(0, 1)].ap())
            nc.gpsimd.dma_start(uh[1][:, 0:2048], uhD[(1, 0)].ap())
            nc.gpsimd.dma_start(uh[1][:, 2048:4096], uhD[(1, 1)].ap())

            # PE warmup: no DMA deps -> runs right after the framework
            # barrier, ramping the PE p-state before real work lands
            wu_ps = ps.tile([128, 512], f32, tag="ps", bufs=7, name="wu_ps")
            for _ in range(4):
                nc.tensor.matmul(wu_ps[:], wu[:, 0:128], wu[:],
                                 start=True, stop=True)

            # build b1R[p, c] = b1[c]/128 per half (K=1 matmul + copy)
            # so the per-block bias init is a uniform K=128 matmul
            for h in range(HH):
                psB = ps.tile([128, 512], f32, tag="ps", bufs=7,
                              name=f"b1b{h}")
                nc.tensor.matmul(psB[:], one128[:],
                                 cst_sb[:, h * 512:(h + 1) * 512],
                                 start=True, stop=True)
                nc.scalar.activation(b1R[:, h * 512:(h + 1) * 512], psB[:],
                                     AF.Copy)

            def bias_mm(psb, h):
                # psum[r, c] = sum_p b1[h*512+c]/128 = b1[h*512+c]
                nc.tensor.matmul(psb[:], onesQ[:],
                                 b1R[:, h * 512:(h + 1) * 512],
                                 start=True, stop=False)

            def data_mms(psb, b, h):
                for k in range(KX):
                    nc.tensor.matmul(
                        psb[:], xb[b][:, k * 128:(k + 1) * 128],
                        w1h[h][:, k * 512:(k + 1) * 512],
                        start=False, stop=(k == KX - 1))

            def dve_chain(b, h, acc_from, acc_to):
                # acc_to[:, b] = reduce_h(hr_half * U_half) + acc_from
                scr = scrp.tile([128, 512], bf16, tag="scr",
                                name=f"scr{h}_{b}")
                nc.vector.tensor_tensor(
                    scr[:], hr[b][:, h * 512:(h + 1) * 512],
                    uh[h][:, b * 512:(b + 1) * 512], OP.mult)
                red = scrp.tile([128, 1], f32, tag="red", name=f"red{h}_{b}")
                nc.vector.tensor_reduce(
                    red[:], scr[:], mybir.AxisListType.X, OP.add)
                nc.vector.tensor_tensor(
                    acc_to[:, b:b + 1], red[:], acc_from, OP.add)

            # sweep 1 (hidden half 0). Bias MMs 0-3 are emitted first --
            # as K=1 matmuls off cst directly (no b1R dependency), they
            # fill the DMA lead-in window; the K=1 pipeline bubble is
            # free there. Later bias inits use the uniform K=128 form.
            ps1 = [ps.tile([128, 512], f32, tag="ps", bufs=7,
                           name=f"ps0_{b}") for b in range(NB)]
            ones_ap = cst_sb[:, HIDDEN:HIDDEN + 128]
            for b in range(4):
                nc.tensor.matmul(ps1[b][:], ones_ap,
                                 cst_sb[:, 0:512],
                                 start=True, stop=False)
            for b in range(NB):
                if b + 4 < NB:
                    bias_mm(ps1[b + 4], 0)
                data_mms(ps1[b], b, 0)
                nc.scalar.activation(hr[b][:, 0:512], ps1[b][:], AF.Relu)
                dve_chain(b, 0, gt_sb[:, b:b + 1], ra)

            # sweep 2 (hidden half 1): per-block bias+data+relu then the
            # closing DVE chain adds the h0 partial
            for b in range(NB - 1):
                psb = ps.tile([128, 512], f32, tag="ps", bufs=7,
                              name=f"ps1_{b}")
                bias_mm(psb, 1)
                data_mms(psb, b, 1)
                nc.scalar.activation(hr[b][:, 512:1024], psb[:], AF.Relu)
                dve_chain(b, 1, ra[:, b:b + 1], outp)

            # last block runs as two 256-wide half-groups so the closing
            # act/mult/reduce pipeline overlaps the final matmuls
            b = NB - 1
            t7 = wgt.tile([128, 1], f32, tag="t7")
            psq2 = psh.tile([128, 512], f32, tag="psq", name="psq")
            for hf in range(2):
                c0 = hf * 256
                psq = psq2[:, c0:c0 + 256]
                nc.tensor.matmul(psq, onesQ[:],
                                 b1R[:, 512 + c0:512 + c0 + 256],
                                 start=True, stop=False)
                for k in range(KX):
                    nc.tensor.matmul(
                        psq, xb[b][:, k * 128:(k + 1) * 128],
                        w1h[1][:, k * 512 + c0:k * 512 + c0 + 256],
                        start=False, stop=(k == KX - 1))
                nc.scalar.activation(hr[b][:, 512 + c0:512 + c0 + 256],
                                     psq, AF.Relu)
                scr = scrp.tile([128, 256], bf16, tag="scrq",
                                name=f"scrq{hf}")
                nc.vector.tensor_tensor(
                    scr[:], hr[b][:, 512 + c0:512 + c0 + 256],
                    uh[1][:, b * 512 + c0:b * 512 + c0 + 256], OP.mult)
                red = scrp.tile([128, 1], f32, tag="redq", name=f"redq{hf}")
                nc.vector.tensor_reduce(
                    red[:], scr[:], mybir.AxisListType.X, OP.add)
                if hf == 0:
                    nc.vector.tensor_tensor(
                        t7[:], ra[:, b:b + 1], red[:], OP.add)
                else:
                    nc.vector.tensor_tensor(
                        outp[:, b:b + 1], t7[:], red[:], OP.add)

            nc.scalar.dma_start(outD.ap(), outp[:])

    nc.compile()
    return nc


def _get_nc():
    if "nc" not in _NC_CACHE:
        import concourse.bacc as bacc
        import concourse.mybir as mybir
        from concourse import tile
        _NC_CACHE["nc"] = _build(bacc.Bacc, mybir, tile)
    return _NC_CACHE["nc"]


def kernel(x_samples, y_idx, W1, b1, W2, b2):
    import ml_dtypes
    from concourse.bass_utils import run_bass_kernel_spmd

    bf16 = ml_dtypes.bfloat16
    x = np.ascontiguousarray(np.asarray(x_samples, dtype=np.float32))
    y = np.asarray(y_idx).astype(np.int64).reshape(-1)
    W1 = np.ascontiguousarray(np.asarray(W1, dtype=np.float32))
    b1 = np.asarray(b1, dtype=np.float32).reshape(-1)
    W2 = np.ascontiguousarray(np.asarray(W2, dtype=np.float32))
    b2 = np.asarray(b2, dtype=np.float32).reshape(-1)

    # global label histogram; fold the softmax-cancelled negative term
    c = np.bincount(y, minlength=Y_DIM).astype(np.float64)
    v = (W2.astype(np.float64) @ c / N).astype(np.float32)     # [H]
    beta = np.float32((b2.astype(np.float64) @ c) / N)
    g_full = (b2[y] - beta).astype(np.float32)                 # [N]

    # U columns, transposed: URt[i, :] = W2[:, y_i] - v
    W2pT = np.ascontiguousarray(W2.T - v[None, :])             # [Y, H]
    W2pT_bf = W2pT.astype(bf16)

    # W1 device layout (shared across cores): h-half 0 per k-chunk
    # (plain row slices), h-half 1 packed [p, k*512+c]
    W1_bf = W1.astype(bf16)
    w1k = [np.ascontiguousarray(W1_bf[k * 128:(k + 1) * 128, 0:512])
           for k in range(KX)]
    w1c = np.ascontiguousarray(
        W1_bf[:, 512:1024].reshape(KX, 128, 512)
        .transpose(1, 0, 2).reshape(128, 2048))
    cst = np.concatenate(
        [b1, np.ones(128, np.float32)]).astype(bf16).reshape(1, -1)

    x_bf = x.astype(bf16)
    in_maps = []
    for m in range(N_CORES):
        sl = slice(m * N_LOC, (m + 1) * N_LOC)
        y_loc = y[sl]
        ur = W2pT_bf[y_loc]                                    # [1024, H]
        im = {"w1c": w1c, "cst": cst,
              "gt": np.ascontiguousarray(
                  g_full[sl].reshape(NB, 128).T)}
        for k in range(KX):
            im[f"w1k{k}"] = w1k[k]
        # xs[p, b*512 + k*128 + r] = x[row0 + b*128 + r, k*128 + p]
        xs = np.ascontiguousarray(
            x_bf[sl].reshape(NB, 128, KX, 128)
            .transpose(3, 0, 2, 1).reshape(128, 4096))
        im["xq0"] = np.ascontiguousarray(xs[:, 0:512])
        im["xq12"] = np.ascontiguousarray(xs[:, 512:1536])
        im["xq345"] = np.ascontiguousarray(xs[:, 1536:3072])
        im["xq67"] = np.ascontiguousarray(xs[:, 3072:4096])
        # uh{hh}{i}[p, b*512 + c] = U[hh*512 + c, row0 + b*128 + p]
        for hh in range(HH):
            uu = np.ascontiguousarray(
                ur[:, hh * 512:(hh + 1) * 512].reshape(NB, 128, 512)
                .transpose(1, 0, 2).reshape(128, 4096))
            im[f"u{hh}0"] = np.ascontiguousarray(uu[:, 0:2048])
            im[f"u{hh}1"] = np.ascontiguousarray(uu[:, 2048:4096])
        in_maps.append(im)

    nc = _get_nc()
    res = run_bass_kernel_spmd(nc, in_maps, core_ids=list(range(N_CORES)))
    # out[p, blk] holds row blk*128+p of the core's 1024 rows
    return np.concatenate(
        [res.results[m]["out"].T.reshape(-1) for m in range(N_CORES)]
    ).astype(np.float32)


# revision 25
# speedup vs baseline: 1.2795x; 1.0069x over previous
"""Trainium2 Bass kernel for nn_CLUBCategorical (CLUB categorical loss).

Reference computation:
    h      = relu(x @ W1 + b1)              [N, H]
    logits = h @ W2 + b2                    [N, Y]
    logp   = log_softmax(logits, -1)        [N, Y]
    out[i] = logp[i, y_i] - mean_j logp[i, y_j]

Algebra: the log-softmax normalizer cancels between the positive and
negative terms, and with c[y] = histogram(y_idx), v = W2 @ c / N:

    out[i] = h_i . (W2[:, y_i] - v) + (b2[y_i] - (b2 . c)/N)
           = h_i . U[:, i] + g[i]

so the entire [N, H] x [H, Y] second GEMM collapses to an elementwise
multiply with the host-gathered U plus a free-dim reduction. Per core
(1024 rows) the device work is just the phase-1 GEMM:

    psum[128 rows, 512 h] = b1 (K=1 ones-matmul) + sum_k xT_blk @ W1_k
    hr = relu(psum)                     (scalar engine, bf16 out)
    delta[128,1] = reduce_h(hr * U_b) + g_b   (one fused DVE
                   tensor_tensor_reduce per 128-row block)

All matmul/elementwise operands are bf16 (PE runs 1 col/cycle at fp32r
and bf16 alike, but bf16 halves DMA to 4MB/core; tolerance is 2e-2 and
bf16 end-to-end lands ~5e-3). Rows are data-parallel across 8 cores; the
"all-gather of column labels" is folded into c/U/g on the host. No
collectives.

Schedule: the 8 sweep-1 bias matmuls are emitted before any data matmul
so the PE ramps its clock and does useful work during the DMA lead-in.
DMA rides two HWDGE queues (sync: cst/W1/U0-3/g, vector: x-blocks/U4-7)
ordered in consumption order.
"""

import numpy as np

N, X_DIM, Y_DIM, HIDDEN = 8192, 512, 512, 1024
N_CORES = 8
N_LOC = N // N_CORES          # 1024 rows per core
NB = N_LOC // 128             # 8 row blocks of 128
KX = X_DIM // 128             # 4 k-chunks
HH = 2                        # two 512-wide hidden halves

_NC_CACHE = {}


def _build(nc_cls, mybir, tile):
    mdt = mybir.dt
    f32 = mdt.float32
    bf16 = mdt.bfloat16
    AF = mybir.ActivationFunctionType
    OP = mybir.AluOpType

    nc = nc_cls("TRN2", target_bir_lowering=False, debug=False,
                num_devices=N_CORES)

    # DRAM tensors (all contiguous, one DMA descriptor each), sized so
    # each queue delivers in exact consumption order
    xq0D = nc.dram_tensor("xq0", [128, 512], bf16, kind="ExternalInput")
    xq12D = nc.dram_tensor("xq12", [128, 1024], bf16, kind="ExternalInput")
    xq345D = nc.dram_tensor("xq345", [128, 1536], bf16,
                            kind="ExternalInput")
    xq67D = nc.dram_tensor("xq67", [128, 1024], bf16, kind="ExternalInput")
    # w1 h-half 0 split per k-chunk; h-half 1 in one block
    w1kD = [nc.dram_tensor(f"w1k{k}", [128, 512], bf16,
                           kind="ExternalInput") for k in range(KX)]
    w1cD = nc.dram_tensor("w1c", [128, 2048], bf16, kind="ExternalInput")
    # U split by hidden half (h0 needed much earlier than h1)
    uhD = {(hh, i): nc.dram_tensor(f"u{hh}{i}", [128, 2048], bf16,
                                   kind="ExternalInput")
           for hh in range(HH) for i in range(2)}
    cstD = nc.dram_tensor("cst", [1, HIDDEN + 128], bf16,
                          kind="ExternalInput")   # [b1 | ones128]
    gtD = nc.dram_tensor("gt", [128, NB], f32, kind="ExternalInput")
    outD = nc.dram_tensor("out", [128, NB], f32, kind="ExternalOutput")

    with tile.TileContext(nc) as tc:
        with (
            tc.tile_pool(name="wgt", bufs=1) as wgt,
            tc.tile_pool(name="scrp", bufs=2) as scrp,
            tc.tile_pool(name="ps", bufs=8, space="PSUM") as ps,
        ):
            cst_sb = wgt.tile([1, HIDDEN + 128], bf16, tag="cst")
            gt_sb = wgt.tile([128, NB], f32, tag="gt")
            wu = wgt.tile([128, 512], bf16, tag="wu")
            w1h = [wgt.tile([128, 2048], bf16, tag=f"w1h{h}", name=f"w1h{h}")
                   for h in range(HH)]
            xsb = wgt.tile([128, 4096], bf16, tag="xsb")
            # U by hidden half: uh[hh][:, b*512:(b+1)*512] = U half for blk b
            uh = [wgt.tile([128, 4096], bf16, tag=f"uh{h}", name=f"uh{h}")
                  for h in range(HH)]
            hr = [wgt.tile([128, 1024], bf16, tag=f"hr{b}", name=f"hr{b}")
                  for b in range(NB)]
            ra = wgt.tile([128, NB], f32, tag="ra")   # h0 partial + g
            outp = wgt.tile([128, NB], f32, tag="outp")
            xb = [xsb[:, b * 512:(b + 1) * 512] for b in range(NB)]

            onesQ = wgt.tile([128, 128], bf16, tag="onesQ")
            b1R = wgt.tile([128, 1024], bf16, tag="b1R")

            one128 = wgt.tile([1, 128], bf16, tag="one128")
            # DVE: warmup source + bias-broadcast constants (no DMA deps)
            nc.vector.memset(wu[:], 0.5)
            nc.vector.memset(one128[:], 1.0 / 128.0)
            nc.vector.memset(onesQ[:], 1.0)

            # All DMA queues share ~320GB/s with racy arbitration, so the
            # layout keeps non-critical bytes (U, 2MB) behind the
            # PE-critical stream instead of competing with it.
            # scalar queue (q10): the PE-critical stream in exact
            # consumption order; out rides it at the end
            nc.scalar.dma_start(xsb[:, 0:512], xq0D.ap())
            nc.scalar.dma_start(w1h[0][:, 0:512], w1kD[0].ap())
            nc.scalar.dma_start(w1h[0][:, 1024:1536], w1kD[2].ap())
            nc.scalar.dma_start(xsb[:, 512:1536], xq12D.ap())
            nc.scalar.dma_start(xsb[:, 1536:3072], xq345D.ap())
            nc.scalar.dma_start(xsb[:, 3072:4096], xq67D.ap())
            nc.scalar.dma_start(w1h[1][:], w1cD.ap())

            # sync queue (q1, otherwise idle): cst first -- the bias MMs
            # need it earliest -- then the tiny g vector
            nc.sync.dma_start(cst_sb[:], cstD.ap())
            nc.sync.dma_start(gt_sb[:], gtD.ap())

            # gpsimd SWDGE: w1-h0 k1/k3 (early, small) then U halves.
            # The dummy copy makes the U triggers wait for the x stream --
            # without it the 2MB U burst starves the PE-critical bytes
            # (queues share bandwidth with racy arbitration).
            nc.gpsimd.dma_start(w1h[0][:, 512:1024], w1kD[1].ap())
            nc.gpsimd.dma_start(w1h[0][:, 1536:2048], w1kD[3].ap())
            # The scheduler orders by data deps only, so gate each U DMA
            # behind the x stream via a WAW stub write into its
            # destination: U-h0 waits for xq12, U-h1 for xq67.
            nc.gpsimd.tensor_copy(uh[0][:, 0:1], xsb[:, 1535:1536])
            nc.gpsimd.tensor_copy(uh[0][:, 2048:2049], xsb[:, 1535:1536])
            nc.gpsimd.tensor_copy(uh[1][:, 0:1], xsb[:, 4095:4096])
            nc.gpsimd.tensor_copy(uh[1][:, 2048:2049], xsb[:, 4095:4096])
            nc.gpsimd.dma_start(uh[0][:, 0:2048], uhD[(0, 0)].ap())
            nc.gpsimd.dma_start(uh[0][:, 2048:4096], uhD[(0, 1)].ap())
            nc.gpsimd.dma_start(uh[1][:, 0:2048], uhD[(1, 0)].ap())
            nc.gpsimd.dma_start(uh[1][:, 2048:4096], uhD[(1, 1)].ap())

            # PE warmup: no DMA deps -> runs right after the framework
            # barrier, ramping the PE p-state before real work lands
            wu_ps = ps.tile([128, 512], f32, tag="ps", bufs=8, name="wu_ps")
            for _ in range(4):
                nc.tensor.matmul(wu_ps[:], wu[:, 0:128], wu[:],
                                 start=True, stop=True)

            # build b1R[p, c] = b1[c]/128 per half (K=1 matmul + copy)
            # so the per-block bias init is a uniform K=128 matmul
            for h in range(HH):
                psB = ps.tile([128, 512], f32, tag="ps", bufs=8,
                              name=f"b1b{h}")
                nc.tensor.matmul(psB[:], one128[:],
                                 cst_sb[:, h * 512:(h + 1) * 512],
                                 start=True, stop=True)
                nc.scalar.activation(b1R[:, h * 512:(h + 1) * 512], psB[:],
                                     AF.Copy)

            def bias_mm(psb, h):
                # psum[r, c] = sum_p b1[h*512+c]/128 = b1[h*512+c]
                nc.tensor.matmul(psb[:], onesQ[:],
                                 b1R[:, h * 512:(h + 1) * 512],
                                 start=True, stop=False)

            def data_mms(psb, b, h):
                for k in range(KX):
                    nc.tensor.matmul(
                        psb[:], xb[b][:, k * 128:(k + 1) * 128],
                        w1h[h][:, k * 512:(k + 1) * 512],
                        start=False, stop=(k == KX - 1))

            def dve_chain(b, h, acc_from, acc_to):
                # acc_to[:, b] = reduce_h(hr_half * U_half) + acc_from
                scr = scrp.tile([128, 512], bf16, tag="scr",
                                name=f"scr{h}_{b}")
                nc.vector.tensor_tensor(
                    scr[:], hr[b][:, h * 512:(h + 1) * 512],
                    uh[h][:, b * 512:(b + 1) * 512], OP.mult)
                red = scrp.tile([128, 1], f32, tag="red", name=f"red{h}_{b}")
                nc.vector.tensor_reduce(
                    red[:], scr[:], mybir.AxisListType.X, OP.add)
                nc.vector.tensor_tensor(
                    acc_to[:, b:b + 1], red[:], acc_from, OP.add)

            # sweep 1 (hidden half 0). Bias MMs 0-3 are emitted first --
            # as K=1 matmuls off cst directly (no b1R dependency), they
            # fill the DMA lead-in window; the K=1 pipeline bubble is
            # free there. Later bias inits use the uniform K=128 form.
            ps1 = [ps.tile([128, 512], f32, tag="ps", bufs=8,
                           name=f"ps0_{b}") for b in range(NB)]
            ones_ap = cst_sb[:, HIDDEN:HIDDEN + 128]
            for b in range(4):
                nc.tensor.matmul(ps1[b][:], ones_ap,
                                 cst_sb[:, 0:512],
                                 start=True, stop=False)
            for b in range(NB):
                if b + 4 < NB:
                    bias_mm(ps1[b + 4], 0)
                data_mms(ps1[b], b, 0)
                nc.scalar.activation(hr[b][:, 0:512], ps1[b][:], AF.Relu)
                dve_chain(b, 0, gt_sb[:, b:b + 1], ra)

            # sweep 2 (hidden half 1): per-block bias+data+relu then the
            # closing DVE chain adds the h0 partial
            for b in range(NB - 1):
                psb = ps.tile([128, 512], f32, tag="ps", bufs=8,
                              name=f"ps1_{b}")
                bias_mm(psb, 1)
                data_mms(psb, b, 1)
                nc.scalar.activation(hr[b][:, 512:1024], psb[:], AF.Relu)
                dve_chain(b, 1, ra[:, b:b + 1], outp)

            # last block runs as two 256-wide half-groups so the closing
            # act/mult/reduce pipeline overlaps the final matmuls
            b = NB - 1
            t7 = wgt.tile([128, 1], f32, tag="t7")
            for hf in range(2):
                c0 = hf * 256
                psq = ps.tile([128, 512], f32, tag="ps", bufs=8,
                              name=f"psq{hf}")[:, 0:256]
                nc.tensor.matmul(psq, onesQ[:],
                                 b1R[:, 512 + c0:512 + c0 + 256],
                                 start=True, stop=False)
                for k in range(KX):
                    nc.tensor.matmul(
                        psq, xb[b][:, k * 128:(k + 1) * 128],
                        w1h[1][:, k * 512 + c0:k * 512 + c0 + 256],
                        start=False, stop=(k == KX - 1))
                nc.scalar.activation(hr[b][:, 512 + c0:512 + c0 + 256],
                                     psq, AF.Relu)
                scr = scrp.tile([128, 256], bf16, tag="scrq",
                                name=f"scrq{hf}")
                nc.vector.tensor_tensor(
                    scr[:], hr[b][:, 512 + c0:512 + c0 + 256],
                    uh[1][:, b * 512 + c0:b * 512 + c0 + 256], OP.mult)
                red = scrp.tile([128, 1], f32, tag="redq", name=f"redq{hf}")
                nc.vector.tensor_reduce(
                    red[:], scr[:], mybir.AxisListType.X, OP.add)
                if hf == 0:
                    nc.vector.tensor_tensor(
                        t7[:], ra[:, b:b + 1], red[:], OP.add)
                else:
                    nc.vector.tensor_tensor(
                        outp[:, b:b + 1], t7[:], red[:], OP.add)

            nc.scalar.dma_start(outD.ap(), outp[:])

    nc.compile()
    return nc


def _get_nc():
    if "nc" not in _NC_CACHE:
        import concourse.bacc as bacc
        import concourse.mybir as mybir
        from concourse import tile
        _NC_CACHE["nc"] = _build(bacc.Bacc, mybir, tile)
    return _NC_CACHE["nc"]


def kernel(x_samples, y_idx, W1, b1, W2, b2):
    import ml_dtypes
    from concourse.bass_utils import run_bass_kernel_spmd

    bf16 = ml_dtypes.bfloat16
    x = np.ascontiguousarray(np.asarray(x_samples, dtype=np.float32))
    y = np.asarray(y_idx).astype(np.int64).reshape(-1)
    W1 = np.ascontiguousarray(np.asarray(W1, dtype=np.float32))
    b1 = np.asarray(b1, dtype=np.float32).reshape(-1)
    W2 = np.ascontiguousarray(np.asarray(W2, dtype=np.float32))
    b2 = np.asarray(b2, dtype=np.float32).reshape(-1)

    # global label histogram; fold the softmax-cancelled negative term
    c = np.bincount(y, minlength=Y_DIM).astype(np.float64)
    v = (W2.astype(np.float64) @ c / N).astype(np.float32)     # [H]
    beta = np.float32((b2.astype(np.float64) @ c) / N)
    g_full = (b2[y] - beta).astype(np.float32)                 # [N]

    # U columns, transposed: URt[i, :] = W2[:, y_i] - v
    W2pT = np.ascontiguousarray(W2.T - v[None, :])             # [Y, H]
    W2pT_bf = W2pT.astype(bf16)

    # W1 device layout (shared across cores): h-half 0 per k-chunk
    # (plain row slices), h-half 1 packed [p, k*512+c]
    W1_bf = W1.astype(bf16)
    w1k = [np.ascontiguousarray(W1_bf[k * 128:(k + 1) * 128, 0:512])
           for k in range(KX)]
    w1c = np.ascontiguousarray(
        W1_bf[:, 512:1024].reshape(KX, 128, 512)
        .transpose(1, 0, 2).reshape(128, 2048))
    cst = np.concatenate(
        [b1, np.ones(128, np.float32)]).astype(bf16).reshape(1, -1)

    x_bf = x.astype(bf16)
    in_maps = []
    for m in range(N_CORES):
        sl = slice(m * N_LOC, (m + 1) * N_LOC)
        y_loc = y[sl]
        ur = W2pT_bf[y_loc]                                    # [1024, H]
        im = {"w1c": w1c, "cst": cst,
              "gt": np.ascontiguousarray(
                  g_full[sl].reshape(NB, 128).T)}
        for k in range(KX):
            im[f"w1k{k}"] = w1k[k]
        # xs[p, b*512 + k*128 + r] = x[row0 + b*128 + r, k*128 + p]
        xs = np.ascontiguousarray(
            x_bf[sl].reshape(NB, 128, KX, 128)
            .transpose(3, 0, 2, 1).reshape(128, 4096))
        im["xq0"] = np.ascontiguousarray(xs[:, 0:512])
        im["xq12"] = np.ascontiguousarray(xs[:, 512:1536])
        im["xq345"] = np.ascontiguousarray(xs[:, 1536:3072])
        im["xq67"] = np.ascontiguousarray(xs[:, 3072:4096])
        # uh{hh}{i}[p, b*512 + c] = U[hh*512 + c, row0 + b*128 + p]
        for hh in range(HH):
            uu = np.ascontiguousarray(
                ur[:, hh * 512:(hh + 1) * 512].reshape(NB, 128, 512)
                .transpose(1, 0, 2).reshape(128, 4096))
            im[f"u{hh}0"] = np.ascontiguousarray(uu[:, 0:2048])
            im[f"u{hh}1"] = np.ascontiguousarray(uu[:, 2048:4096])
        in_maps.append(im)

    nc = _get_nc()
    res = run_bass_kernel_spmd(nc, in_maps, core_ids=list(range(N_CORES)))
    # out[p, blk] holds row blk*128+p of the core's 1024 rows
    return np.concatenate(
        [res.results[m]["out"].T.reshape(-1) for m in range(N_CORES)]
    ).astype(np.float32)
